# revision 2
# baseline (speedup 1.0000x reference)
"""2-layer GraphSAGE (PyG SAGEConv mean-aggregation) on 8 trn2 NeuronCores.

Contract: kernel(**inputs) takes the FULL unsharded inputs
(x [100000,128] f32, edge_index [2,1600000] i32, W1_l/W1_r/W2_l/W2_r
[128,128] f32, b1/b2 [128] f32) and returns the FULL [100000,128] f32
output.

Distribution: dst-node blocks of 256 are dealt to the 8 cores balanced by
in-edge count; each core aggregates its own dst set. Per 128-edge subtile
the kernel dma_gathers source rows (512B) with int16 chunk-local indices,
builds a one-hot [128e x 256dst] selector on VectorE and matmul-accumulates
(fp32r) into the block's PSUM tile, giving the segment sum in [D, 256dst]
layout. Per 128-dst half block: W_l matmul on the mean columns, W_r matmul
on host-transposed x columns (layer 1) / PE-transposed h (layer 2); 1/deg
is applied after the W_l matmul as a per-partition scalar. h shards are
AllGather'd between the layers for the layer-2 gathers.
"""
import sys

for _p in ("/opt/trn_rl_repo", "/root/.axon_site/_ro/trn_rl_repo"):
    if _p not in sys.path:
        sys.path.append(_p)

import numpy as np

import concourse.bacc as bacc
import concourse.mybir as mybir
from concourse.tile import TileContext
from concourse.bass_utils import run_bass_kernel_spmd

F32 = mybir.dt.float32
F32R = mybir.dt.float32r
I16 = mybir.dt.int16

P = 8          # cores
D = 128        # feature dim
BW = 256       # dst block width
CHUNK = 32768  # int16-addressable gather window (rows)
GB = 2         # block-slots per gather group
SENT = 300.0   # one-hot sentinel (never matches iota 0..255)


def split_multiwaits(nc, max_waits=1):
    """walrus rejects instructions carrying several semaphore waits; hoist
    excess waits onto single-wait NOPs inserted just before."""
    n_split = 0
    for bb in nc.main_func.blocks:
        i = 0
        instrs = bb.instructions
        while i < len(instrs):
            ins = instrs[i]
            si = ins.sync_info
            if si is not None and len(si.on_wait) > max_waits:
                waits = list(si.on_wait)
                spill, keep = waits[:-max_waits], waits[-max_waits:]
                for j, w in enumerate(spill):
                    nop = mybir.InstNoOp(name=f"{ins.name}_wsplit{j}", ins=[], outs=[])
                    nop.engine = ins.engine
                    nop.sync_info = mybir.SyncInfo(on_wait=[w], on_update=[])
                    nc.register_instruction(nop, overwrite=True)
                    instrs.insert(i, nop)
                    i += 1
                si.on_wait = keep
                n_split += 1
            i += 1
    return n_split


# ---------------------------------------------------------------- host side
def plan_blocks(dst, n_nodes):
    ngb = -(-n_nodes // BW)
    nslot = -(-ngb // P)
    w = np.bincount(dst // BW, minlength=ngb)
    order = np.argsort(-w, kind="stable")
    order = np.concatenate([order, -np.ones(nslot * P - ngb, np.int64)])
    core_blocks = np.empty((P, nslot), np.int64)
    for s in range(nslot):
        grp = order[s * P:(s + 1) * P]
        for c in range(P):
            core_blocks[c, s] = grp[c]
    owner = np.full(ngb, -1, np.int64)
    slot_of = np.full(ngb, -1, np.int64)
    for c in range(P):
        for s in range(nslot):
            g = core_blocks[c, s]
            if g >= 0:
                owner[g] = c
                slot_of[g] = s
    return core_blocks, owner, slot_of, ngb, nslot


def schedule_layer(dst, rowidx, owner, slot_of, nslot, nrows):
    """Static SPMD schedule + per-core idx/slot arrays for one layer."""
    nch = -(-nrows // CHUNK)
    blk = dst // BW
    ecore = owner[blk]
    eslot = slot_of[blk]
    echunk = rowidx // CHUNK
    eslotin = (dst % BW).astype(np.float32)

    C = np.zeros((P, nslot, nch), np.int64)
    np.add.at(C, (ecore, eslot, echunk), 1)
    Q = np.maximum(-(-C.max(axis=0) // 128), 0)
    Q[:, 0] = np.maximum(Q[:, 0], 1)   # ensure PSUM init per slot

    ngrp = -(-nslot // GB)
    groups = [list(range(g * GB, min((g + 1) * GB, nslot))) for g in range(ngrp)]

    bucket_sub0 = np.zeros((nslot, nch), np.int64)
    call_info = []
    grp_sub0 = []
    t = 0
    for g, gs in enumerate(groups):
        grp_sub0.append(t)
        for q in range(nch):
            c0 = t
            for s in gs:
                bucket_sub0[s, q] = t
                t += Q[s, q]
            call_info.append((g, q, c0, t - c0))
    nsubt = t
    nidxt = nsubt * 128

    idx_all = np.zeros((P, nidxt), np.int16)
    slot_all = np.full((P, nidxt), SENT, np.float32)
    eorder = np.lexsort((echunk, eslot, ecore))
    key = (ecore * nslot + eslot) * nch + echunk
    key_sorted = key[eorder]
    starts = np.searchsorted(key_sorted, np.arange(P * nslot * nch))
    ends = np.searchsorted(key_sorted, np.arange(P * nslot * nch) + 1)
    lidx = (rowidx - echunk * CHUNK).astype(np.int16)
    for c in range(P):
        for s in range(nslot):
            for q in range(nch):
                k = (c * nslot + s) * nch + q
                a, b = starts[k], ends[k]
                if a == b:
                    continue
                es = eorder[a:b]
                o = bucket_sub0[s, q] * 128
                idx_all[c, o:o + (b - a)] = lidx[es]
                slot_all[c, o:o + (b - a)] = eslotin[es]

    idx_wrapped = np.empty((P, 128, nidxt // 16), np.int16)
    for (g, q, c0, ns) in call_info:
        if ns == 0:
            continue
        seg = idx_all[:, c0 * 128:(c0 + ns) * 128]
        w = seg.reshape(P, -1, 16).transpose(0, 2, 1)
        idx_wrapped[:, :, c0 * 8:(c0 + ns) * 8] = np.tile(w, (1, 8, 1))
    slot_cols = slot_all.reshape(P, nsubt, 128).transpose(0, 2, 1).copy()

    slot_subs = {}
    for g, gs in enumerate(groups):
        for s in gs:
            subs = []
            for q in range(nch):
                b0 = bucket_sub0[s, q]
                for u in range(Q[s, q]):
                    subs.append(b0 + u)
            slot_subs[s] = [(u, u - grp_sub0[g]) for u in subs]

    chunk_bounds = [(q * CHUNK, min((q + 1) * CHUNK, nrows)) for q in range(nch)]
    return dict(nch=nch, Q=Q, groups=groups, call_info=call_info,
                grp_sub0=grp_sub0, nsubt=nsubt, nidxt=nidxt,
                idx=idx_wrapped, slot=slot_cols, slot_subs=slot_subs,
                chunk_bounds=chunk_bounds,
                grp_nsub=[(grp_sub0[g + 1] if g + 1 < len(grp_sub0) else nsubt)
                          - grp_sub0[g] for g in range(len(groups))])


def preprocess(edge_index, n_nodes):
    src = edge_index[0].astype(np.int64)
    dst = edge_index[1].astype(np.int64)
    core_blocks, owner, slot_of, ngb, nslot = plan_blocks(dst, n_nodes)
    s_pad = nslot * BW
    hall_rows = P * s_pad

    plan1 = schedule_layer(dst, src, owner, slot_of, nslot, n_nodes)

    nodes = np.arange(n_nodes, dtype=np.int64)
    nblk = nodes // BW
    pi = owner[nblk] * s_pad + slot_of[nblk] * BW + (nodes % BW)
    plan2 = schedule_layer(dst, pi[src], owner, slot_of, nslot, hall_rows)

    dst_ids = np.full((P, s_pad), -1, np.int64)
    for c in range(P):
        for s in range(nslot):
            g = core_blocks[c, s]
            if g < 0:
                continue
            ids = g * BW + np.arange(BW)
            ids[ids >= n_nodes] = -1
            dst_ids[c, s * BW:(s + 1) * BW] = ids

    deg = np.bincount(dst, minlength=n_nodes).astype(np.float32)
    deg = np.maximum(deg, 1.0)
    cnt = np.ones((P, 128, 2 * nslot), np.float32)
    for c in range(P):
        ids = dst_ids[c]
        v = np.where(ids >= 0, deg[np.clip(ids, 0, n_nodes - 1)], 1.0)
        cnt[c] = v.reshape(2 * nslot, 128).T
    return dict(nslot=nslot, s_pad=s_pad, hall_rows=hall_rows,
                dst_ids=dst_ids, cnt=cnt, plan1=plan1, plan2=plan2)


# ------------------------------------------------------------- device side
def emit_layer(nc, pools, plan, src_dram, idx_dram, slot_dram,
               wlT_t, wrT_t, brow_t, iota_t, recip_t, identity_t,
               xT_dram, hT_sb_in, h_shard, hT_sb_out, out_dram, relu,
               add_bias, bias_ones_t):
    mpool, spool, wpool, ppA, ppL, ppR, ppT, ipool = pools
    groups = plan["groups"]
    grp_sub0 = plan["grp_sub0"]
    max_gn = max(plan["grp_nsub"])

    idx_t = ipool.tile([128, plan["nidxt"] // 16], I16, tag="idx", name="idx_t")
    nc.sync.dma_start(out=idx_t[:], in_=idx_dram[:])
    slot_t = ipool.tile([128, plan["nsubt"]], F32, tag="slot", name="slot_t")
    nc.sync.dma_start(out=slot_t[:], in_=slot_dram[:])

    calls_by_grp = {}
    for (g, q, c0, ns) in plan["call_info"]:
        calls_by_grp.setdefault(g, []).append((q, c0, ns))

    for g, gs in enumerate(groups):
        msg = mpool.tile([128, max_gn * 128], F32R, tag="msg", name="msg")
        g0 = grp_sub0[g]
        for (q, c0, ns) in calls_by_grp[g]:
            if ns == 0:
                continue
            lo, hi = plan["chunk_bounds"][q]
            ni = ns * 128
            nc.gpsimd.dma_gather(
                msg[:, (c0 - g0) * 128:(c0 - g0 + ns) * 128]
                    .rearrange("p (t e) -> p t e", e=D),
                src_dram[lo:hi, :].bitcast(F32R),
                idx_t[:, c0 * 8:(c0 + ns) * 8],
                ni, ni, D,
                single_packet=(ni <= 1024),
            )
        for s in gs:
            psA = ppA.tile([128, BW], F32, space="PSUM", tag="agg", name="psA")
            subs = plan["slot_subs"][s]
            for i, (u, lu) in enumerate(subs):
                sp = spool.tile([128, BW], F32R, tag="sp", name="sp")
                nc.vector.tensor_scalar(
                    out=sp[:], in0=iota_t[:],
                    scalar1=slot_t[:, u:u + 1], scalar2=None,
                    op0=mybir.AluOpType.is_equal,
                )
                nc.tensor.matmul(
                    out=psA[:],
                    lhsT=msg[:, lu * 128:(lu + 1) * 128],
                    rhs=sp[:],
                    start=(i == 0), stop=(i == len(subs) - 1),
                )
            mean_sb = wpool.tile([128, BW], F32R, tag="mean", name="mean_sb")
            nc.scalar.activation(mean_sb[:], psA[:],
                                 mybir.ActivationFunctionType.Copy)
            for hf in range(2):
                j = 2 * s + hf
                psL = ppL.tile([128, 128], F32, space="PSUM", tag="lin_l", name="psL")
                nc.tensor.matmul(out=psL[:],
                                 lhsT=mean_sb[:, hf * 128:(hf + 1) * 128],
                                 rhs=wlT_t[:], start=True, stop=True)
                psR = ppR.tile([128, 128], F32, space="PSUM", tag="lin_r", name="psR")
                if xT_dram is not None:
                    xT_blk = wpool.tile([128, 128], F32R, tag="xT", name="xT_blk")
                    nc.sync.dma_start(
                        out=xT_blk[:],
                        in_=xT_dram[:, j * 128:(j + 1) * 128].bitcast(F32R))
                    rlhs = xT_blk[:]
                else:
                    rlhs = hT_sb_in[:, j * 128:(j + 1) * 128]
                nc.tensor.matmul(out=psR[:], lhsT=rlhs, rhs=wrT_t[:],
                                 start=True, stop=not add_bias)
                if add_bias:
                    nc.tensor.matmul(out=psR[:], lhsT=bias_ones_t[:],
                                     rhs=brow_t[:], start=False, stop=True)
                tmp = wpool.tile([128, 128], F32, tag="tmp", name="tmp")
                nc.vector.tensor_scalar(out=tmp[:], in0=psL[:],
                                        scalar1=recip_t[:, j:j + 1], scalar2=None,
                                        op0=mybir.AluOpType.mult)
                sum_sb = wpool.tile([128, 128], F32, tag="sum", name="sum_sb")
                nc.vector.tensor_tensor(out=sum_sb[:], in0=tmp[:], in1=psR[:],
                                        op=mybir.AluOpType.add)
                if relu:
                    h_sb = wpool.tile([128, 128], F32, tag="h", name="h_sb")
                    nc.scalar.activation(h_sb[:], sum_sb[:],
                                         mybir.ActivationFunctionType.Relu)
                    nc.sync.dma_start(out=h_shard[j * 128:(j + 1) * 128, :],
                                      in_=h_sb[:])
                    psT = ppT.tile([128, 128], F32, space="PSUM", tag="tr", name="psT")
                    nc.tensor.transpose(psT[:], h_sb[:], identity_t[:])
                    nc.scalar.activation(hT_sb_out[:, j * 128:(j + 1) * 128], psT[:],
                                         mybir.ActivationFunctionType.Copy)
                else:
                    nc.sync.dma_start(out=out_dram[j * 128:(j + 1) * 128, :],
                                      in_=sum_sb[:])


def build_program(pre, n_nodes, add_bias, iters=1, timing_mode=False):
    nslot = pre["nslot"]
    s_pad = pre["s_pad"]
    p1, p2 = pre["plan1"], pre["plan2"]

    nc = bacc.Bacc("TRN2", target_bir_lowering=False)
    ein = {}
    ein["x"] = nc.declare_dram_parameter("x", [n_nodes, D], F32, isOutput=False)
    ein["xT"] = nc.declare_dram_parameter("xT", [D, s_pad], F32, isOutput=False)
    ein["idx1"] = nc.declare_dram_parameter("idx1", [128, p1["nidxt"] // 16], I16,
                                            isOutput=False)
    ein["slot1"] = nc.declare_dram_parameter("slot1", [128, p1["nsubt"]], F32,
                                             isOutput=False)
    ein["idx2"] = nc.declare_dram_parameter("idx2", [128, p2["nidxt"] // 16], I16,
                                            isOutput=False)
    ein["slot2"] = nc.declare_dram_parameter("slot2", [128, p2["nsubt"]], F32,
                                             isOutput=False)
    ein["cnt"] = nc.declare_dram_parameter("cnt", [128, 2 * nslot], F32,
                                           isOutput=False)
    for nm in ("wl1T", "wr1T", "wl2T", "wr2T"):
        ein[nm] = nc.declare_dram_parameter(nm, [D, D], F32, isOutput=False)
    ein["b1row"] = nc.declare_dram_parameter("b1row", [1, D], F32, isOutput=False)
    ein["b2row"] = nc.declare_dram_parameter("b2row", [1, D], F32, isOutput=False)
    ein["iota"] = nc.declare_dram_parameter("iota", [128, BW], F32, isOutput=False)
    ein["ones1"] = nc.declare_dram_parameter("ones1", [1, 128], F32, isOutput=False)
    ein["ident"] = nc.declare_dram_parameter("ident", [128, 128], F32, isOutput=False)
    out_dram = nc.declare_dram_parameter("out_shard", [s_pad, D], F32, isOutput=True)

    h_shard = nc.dram_tensor("h_shard", [s_pad, D], F32)
    h_all = nc.dram_tensor("h_all", [pre["hall_rows"], D], F32, addr_space="Shared")

    with TileContext(nc) as tc:
        with tc.tile_pool(name="const", bufs=1) as cpool, \
             tc.tile_pool(name="msg", bufs=2) as mpool, \
             tc.tile_pool(name="sp", bufs=4) as spool, \
             tc.tile_pool(name="work", bufs=3) as wpool, \
             tc.tile_pool(name="hTp", bufs=1) as hTp, \
             tc.tile_pool(name="io", bufs=1) as ipool, \
             tc.tile_pool(name="ppA", bufs=2, space="PSUM") as ppA, \
             tc.tile_pool(name="ppL", bufs=2, space="PSUM") as ppL, \
             tc.tile_pool(name="ppR", bufs=2, space="PSUM") as ppR, \
             tc.tile_pool(name="ppT", bufs=2, space="PSUM") as ppT:

            iota_t = cpool.tile([128, BW], F32, name="iota_t")
            nc.sync.dma_start(out=iota_t[:], in_=ein["iota"][:])
            identity_t = cpool.tile([128, 128], F32, name="identity_t")
            nc.sync.dma_start(out=identity_t[:], in_=ein["ident"][:])
            cnt_t = cpool.tile([128, 2 * nslot], F32, name="cnt_t")
            nc.sync.dma_start(out=cnt_t[:], in_=ein["cnt"][:])
            recip_t = cpool.tile([128, 2 * nslot], F32, name="recip_t")
            nc.vector.reciprocal(recip_t[:], cnt_t[:])
            wt = {}
            for nm in ("wl1T", "wr1T", "wl2T", "wr2T"):
                wt[nm] = cpool.tile([D, D], F32R, tag=nm, name=nm)
                nc.sync.dma_start(out=wt[nm][:], in_=ein[nm][:].bitcast(F32R))
            brow = {}
            for nm in ("b1row", "b2row"):
                brow[nm] = cpool.tile([1, D], F32R, tag=nm, name=nm)
                nc.sync.dma_start(out=brow[nm][:], in_=ein[nm][:].bitcast(F32R))
            ones_t = cpool.tile([1, 128], F32R, name="ones_t")
            nc.sync.dma_start(out=ones_t[:], in_=ein["ones1"][:].bitcast(F32R))

            hT_sb = hTp.tile([128, s_pad], F32R, name="hT_sb")

            pools = (mpool, spool, wpool, ppA, ppL, ppR, ppT, ipool)

            def body(_iv=None):
                emit_layer(nc, pools, p1, ein["x"], ein["idx1"], ein["slot1"],
                           wt["wl1T"], wt["wr1T"], brow["b1row"], iota_t,
                           recip_t, identity_t, ein["xT"], None,
                           h_shard, hT_sb, None, True, add_bias, ones_t)
                nc.gpsimd.collective_compute(
                    "AllGather", mybir.AluOpType.bypass,
                    replica_groups=[list(range(P))],
                    ins=[h_shard[:]], outs=[h_all[:]],
                )
                emit_layer(nc, pools, p2, h_all, ein["idx2"], ein["slot2"],
                           wt["wl2T"], wt["wr2T"], brow["b2row"], iota_t,
                           recip_t, identity_t, None, hT_sb,
                           None, None, out_dram, False, add_bias, ones_t)

            def body_l1():
                emit_layer(nc, pools, p1, ein["x"], ein["idx1"], ein["slot1"],
                           wt["wl1T"], wt["wr1T"], brow["b1row"], iota_t,
                           recip_t, identity_t, ein["xT"], None,
                           h_shard, hT_sb, None, True, add_bias, ones_t)

            def body_l2():
                emit_layer(nc, pools, p2, h_all, ein["idx2"], ein["slot2"],
                           wt["wl2T"], wt["wr2T"], brow["b2row"], iota_t,
                           recip_t, identity_t, None, hT_sb,
                           None, None, out_dram, False, add_bias, ones_t)

            if not timing_mode:
                body()
            else:
                # collectives cannot sit inside a Tile For_i on this stack;
                # run layer1+AllGather once, then loop both layers without
                # the collective (the loop delta measures t_l1 + t_l2).
                body_l1()
                nc.gpsimd.collective_compute(
                    "AllGather", mybir.AluOpType.bypass,
                    replica_groups=[list(range(P))],
                    ins=[h_shard[:]], outs=[h_all[:]],
                )
                with tc.For_i(0, iters, 1):
                    body_l1()
                    body_l2()

    nc.compile()
    split_multiwaits(nc, max_waits=1)
    return nc


def make_inputs(pre, x, W1_l, W1_r, b1, W2_l, W2_r, b2):
    s_pad = pre["s_pad"]
    p1, p2 = pre["plan1"], pre["plan2"]
    common = dict(
        wl1T=np.ascontiguousarray(W1_l.T), wr1T=np.ascontiguousarray(W1_r.T),
        wl2T=np.ascontiguousarray(W2_l.T), wr2T=np.ascontiguousarray(W2_r.T),
        b1row=np.asarray(b1, np.float32).reshape(1, -1),
        b2row=np.asarray(b2, np.float32).reshape(1, -1),
        iota=np.tile(np.arange(BW, dtype=np.float32), (128, 1)),
        ones1=np.ones((1, 128), np.float32),
        ident=np.eye(128, dtype=np.float32),
        x=x,
    )
    in_maps = []
    for c in range(P):
        ids = pre["dst_ids"][c]
        xT = np.zeros((D, s_pad), np.float32)
        valid = ids >= 0
        xT[:, valid] = x[ids[valid]].T
        m = dict(common)
        m.update(xT=xT, cnt=pre["cnt"][c],
                 idx1=p1["idx"][c], slot1=p1["slot"][c],
                 idx2=p2["idx"][c], slot2=p2["slot"][c])
        in_maps.append(m)
    return in_maps


def assemble_output(pre, results, n_nodes):
    out = np.zeros((n_nodes, D), np.float32)
    for c in range(P):
        ids = pre["dst_ids"][c]
        shard = results[c]["out_shard"]
        valid = ids >= 0
        out[ids[valid]] = shard[valid]
    return out


_cache = {}


def _get_program(edge_index, n_nodes, add_bias):
    key = (n_nodes, add_bias,
           hash(edge_index.tobytes()) if edge_index.nbytes < (1 << 31) else id(edge_index))
    hit = _cache.get(key)
    if hit is not None:
        return hit
    pre = preprocess(edge_index, n_nodes)
    nc = build_program(pre, n_nodes, add_bias)
    _cache[key] = (pre, nc)
    return pre, nc


def kernel(x, edge_index, W1_l, W1_r, b1, W2_l, W2_r, b2):
    x = np.ascontiguousarray(np.asarray(x, np.float32))
    edge_index = np.ascontiguousarray(np.asarray(edge_index))
    n_nodes = x.shape[0]
    add_bias = bool(np.any(np.asarray(b1)) or np.any(np.asarray(b2)))
    pre, nc = _get_program(edge_index, n_nodes, add_bias)
    in_maps = make_inputs(pre, x, np.asarray(W1_l, np.float32),
                          np.asarray(W1_r, np.float32), b1,
                          np.asarray(W2_l, np.float32),
                          np.asarray(W2_r, np.float32), b2)
    res = run_bass_kernel_spmd(nc, in_maps, list(range(P)))
    return assemble_output(pre, res.results, n_nodes)
